# revision 29
# baseline (speedup 1.0000x reference)
"""AdaIN statistics kernel for TRN2, SPMD across 8 NeuronCores. v8.

Input : f_vol [32, 512, 64, 64] f32
Output: [32, 1024] f32 = concat([mean over (h,w), unbiased std over (h,w)], axis=-1)

Sharding: data-parallel over batch - each of the 8 cores handles 4 batches
([4, 512, 64, 64] shard, 32 MiB). No collectives; the host concatenates the
8 per-core [4, 1024] outputs.

DMA facts established by v2-v7 traces on this part:
  - Per-engine stream rate depends on descriptor size: ~26.5 B/ns at
    32 KiB/partition, ~20.6 at 8 KiB, ~14 with sub-128-partition DMAs.
    So the bulk must ride 32 KiB-descriptor full-128-lane slabs.
  - SDMA engine column 15 runs ~21 B/ns whenever all 16 are loaded (both
    HWDGE queues), so its 1/16 byte share sets the stream end (~106 us).
    No descriptor layout shifts bytes off it without the small-DMA
    penalty (v5), so the kernel accepts its pace.
  - Whole-slab completion semaphores fire only when engine 15 drains the
    slab, so slab-gated compute lags the stream.  Pair-slabs complete at
    ~11.9 us spacing vs ~10.9 us of DVE work per pair - DVE keeps pace
    at slab granularity; only the last two rows need finer chunking so
    the exposed tail is one 128-col bn_stats + bn_aggr + 2 ACT ops + 2
    tiny out-DMAs (mean from scalar, std from sync, in parallel).

Lane-major deal: lane p owns rows 16p..16p+15 (row = b*512+c of the
[2048, 4096] row-major view); slot u = output column 16*(p%32)+u of
batch p//32.  ACT consumes slots 0-5 as three whole pair-slabs
(Copy/Square+accumulate); DVE consumes slots 6-13 as four pair-slabs,
slot 14 as one single-row DMA (16 KiB descriptors, arrives ~8 us before
stream end, stats hidden), slot 15 chunked 2048/1024/512/384/128.

Outputs accumulate in MS[128, 2, 16]; the whole output is 4 DMAs: early
mean/std for slots 0..14 (hidden mid-stream), final mean/std for slot 15.

Semaphore discipline (v2 lesson): same-engine RAW through SBUF is NOT
covered by program order; every producer->consumer edge carries an
explicit semaphore observation.  Semaphores and stats buffers are never
reused.  SBUF reuse: DVE pair d3 lands in ACT slab 0's buffer and rows
14/15 in ACT slab 1's, gated on act_stats (ACT's reads retired).
"""

from contextlib import ExitStack

import numpy as np

B, C, H, W = 32, 512, 64, 64
N_CORES = 8
B_LOCAL = B // N_CORES  # 4
N = H * W  # 4096
P = 128
ROWS = B_LOCAL * C  # 2048
RPL = ROWS // P  # 16 rows (slots) per lane

NACT = 3  # ACT pair-slabs: slots (0,1), (2,3), (4,5)
NDVE = 4  # DVE pair-slabs: slots (6,7), (8,9), (10,11), (12,13)
R15_CHUNKS = [2048, 1024, 512, 384, 128]
GROUP15 = [512] * 7 + [384, 128]
assert sum(R15_CHUNKS) == N and sum(GROUP15) == N

_CACHE = {}


def _build():
    import concourse.bass as bass
    from concourse import mybir

    nc = bass.Bass()
    x_ext = nc.declare_dram_parameter(
        "f_vol", [B_LOCAL, C, H, W], mybir.dt.float32, isOutput=False
    )
    out_ext = nc.declare_dram_parameter(
        "out", [B_LOCAL, 2 * C], mybir.dt.float32, isOutput=True
    )

    xl = (
        x_ext.ap()
        .rearrange("b c h w -> (b c) (h w)")
        .rearrange("(p u) f -> p (u f)", u=RPL)
    )

    # DVE cumulative bn_stats after each unit: pairs d0-d2 16 each; d3 is
    # split (DVE row 12 only, 8 stats - ACT consumes row 13 so DVE's
    # terminal backlog shrinks by ~5 us); r14 8; r15 9.
    D_PAIR = {0: 16, 1: 32, 2: 48, 3: 56}
    D_R14 = 65  # row 14 chunked: 9 groups
    DVE_TOTAL = 74
    # mv_ready: rows 6..12 -> 1..7 (row 13 is acc-form), r14 -> 8, r15 -> 9
    MV = {u: u - 5 for u in range(6, 13)}
    MV[14] = 8
    MV[15] = 9
    # act_stats: acc pairs 0-2 -> 4, 8, 12; row-13 pass -> 14
    ACTS = {a: 4 * (a + 1) for a in range(NACT)}
    ACTS_R13 = 14

    with ExitStack() as ctx:
        block = ctx.enter_context(nc.Block(no_gpsimd_drain=True))
        dma_a = [ctx.enter_context(nc.semaphore(f"dma_a{a}")) for a in range(NACT)]
        dma_d = [ctx.enter_context(nc.semaphore(f"dma_d{k}")) for k in range(NDVE)]
        dma_r14c = [
            ctx.enter_context(nc.semaphore(f"dma_r14c{i}"))
            for i in range(len(R15_CHUNKS))
        ]
        dma_rc = [
            ctx.enter_context(nc.semaphore(f"dma_r15c{i}"))
            for i in range(len(R15_CHUNKS))
        ]
        out_sem = ctx.enter_context(nc.semaphore("out_sem"))
        fin_sem = ctx.enter_context(nc.semaphore("fin_sem"))
        dve_stats = ctx.enter_context(nc.semaphore("dve_stats"))
        mv_ready = ctx.enter_context(nc.semaphore("mv_ready"))
        act_stats = ctx.enter_context(nc.semaphore("act_stats"))
        act_done = ctx.enter_context(nc.semaphore("act_done"))
        warm_done = ctx.enter_context(nc.semaphore("warm_done"))

        xtA = ctx.enter_context(
            nc.sbuf_tensor("xtA", [P, NACT, 2 * N], mybir.dt.float32)
        )
        xtD = ctx.enter_context(nc.sbuf_tensor("xtD", [P, 3, 2 * N], mybir.dt.float32))
        stats = ctx.enter_context(
            nc.sbuf_tensor("stats", [P, 10, 9, 6], mybir.dt.float32)
        )
        mv = ctx.enter_context(nc.sbuf_tensor("mv", [P, 10, 2], mybir.dt.float32))
        MS = ctx.enter_context(nc.sbuf_tensor("MS", [P, 2, RPL], mybir.dt.float32))
        acc = ctx.enter_context(
            nc.sbuf_tensor("acc", [P, NACT + 1, 2, 3], mybir.dt.float32)
        )
        warm = ctx.enter_context(nc.sbuf_tensor("warm", [P, 2], mybir.dt.float32))

        # DVE pair k buffer: k<3 -> xtD slot k; k=3 -> xtA slot 0 (reused).
        # Row 14 -> xtA slot 1 low half; row 15 -> xtA slot 1 high half.
        def dbuf(k):
            return xtD[:, k, :] if k < 3 else xtA[:, 0, :]

        r14_buf = xtA[:, 1, 0:N]
        r15_buf = xtA[:, 1, N : 2 * N]

        # act_done gates, ACT emission order: acc epis 0..2 (8 each), row-13
        # epi (4), mv epis 6..12 (2 each), mv epi 14, early out, mv epi 15
        actd_acc = {a: 8 * (a + 1) for a in range(NACT)}
        ACTD_R13 = 28
        actd_mv = {u: 28 + 2 * (u - 5) for u in range(6, 13)}  # 30..42
        actd_mv[14] = 44
        actd_mv[15] = 46
        ACT_TOTAL = actd_mv[15]

        def out_dma(eng, sem, is_std, u0, w):
            dst = bass.AP(
                tensor=out_ext,
                offset=is_std * C + u0,
                ap=[[2 * C, B_LOCAL], [RPL, P // B_LOCAL], [1, w]],
            )
            with nc.allow_non_contiguous_dma(reason="4B-per-lane stat column"):
                eng.dma_start(out=dst, in_=MS[:, is_std, u0 : u0 + w]).then_inc(
                    sem, 16
                )

        @block.sync
        def _(sync):
            def pair_src(s0):  # slots (s0, s0+1)
                return xl[:, s0 * N : (s0 + 2) * N]

            # interleave ACT and DVE pair-slabs
            sync.dma_start(out=xtA[:, 0, :], in_=pair_src(0)).then_inc(dma_a[0], 16)
            sync.dma_start(out=dbuf(0), in_=pair_src(6)).then_inc(dma_d[0], 16)
            sync.dma_start(out=xtA[:, 1, :], in_=pair_src(2)).then_inc(dma_a[1], 16)
            sync.dma_start(out=dbuf(1), in_=pair_src(8)).then_inc(dma_d[1], 16)
            sync.dma_start(out=xtA[:, 2, :], in_=pair_src(4)).then_inc(dma_a[2], 16)
            sync.dma_start(out=dbuf(2), in_=pair_src(10)).then_inc(dma_d[2], 16)
            # d3 reuses ACT slab 0's buffer; ACT finished reading it at
            # act_stats >= 4
            sync.wait_ge(act_stats, ACTS[0])
            sync.dma_start(out=dbuf(3), in_=pair_src(12)).then_inc(dma_d[3], 16)
            # rows 14/15 reuse ACT slab 1's buffer
            sync.wait_ge(act_stats, ACTS[1])
            c0 = 0
            for i, w in enumerate(R15_CHUNKS):
                sync.dma_start(
                    out=r14_buf[:, c0 : c0 + w],
                    in_=xl[:, 14 * N + c0 : 14 * N + c0 + w],
                ).then_inc(dma_r14c[i], 16)
                c0 += w
            c0 = 0
            for i, w in enumerate(R15_CHUNKS):
                sync.dma_start(
                    out=r15_buf[:, c0 : c0 + w],
                    in_=xl[:, 15 * N + c0 : 15 * N + c0 + w],
                ).then_inc(dma_rc[i], 16)
                c0 += w
            # final std out for slot 15, parallel with scalar's mean out
            sync.wait_ge(act_done, ACT_TOTAL)
            out_dma(sync, fin_sem, 1, RPL - 1, 1)
            sync.wait_ge(out_sem, 16 * 3)
            sync.wait_ge(fin_sem, 16)

        @block.vector
        def _(vector):
            ndve = 0
            nmv = 0

            vector.memset(warm[:, :], 0.0).then_inc(warm_done, 1)

            for k in range(NDVE):
                rows = (0, 1) if k < 3 else (0,)  # d3 row 1 goes to ACT
                vector.wait_ge(dma_d[k], 16)
                buf = dbuf(k)
                for r in rows:
                    si = 2 * k + r  # stats index for slot u = 6+2k+r
                    for g in range(8):
                        vector.bn_stats(
                            out=stats[:, si, g, :],
                            in_=buf[:, (r * 8 + g) * 512 : (r * 8 + g + 1) * 512],
                        ).then_inc(dve_stats, 1)
                        ndve += 1
                assert ndve == D_PAIR[k]
                vector.wait_ge(dve_stats, ndve)
                for r in rows:
                    si = 2 * k + r
                    vector.bn_aggr(
                        out=mv[:, si, :], in_=stats[:, si, 0:8, :]
                    ).then_inc(mv_ready, 1)
                    nmv += 1
                assert nmv == MV[6 + 2 * k + rows[-1]]

            # row 14: chunk-paced like row 15 (9 groups)
            gi = 0
            c0 = 0
            for i, w in enumerate(R15_CHUNKS):
                vector.wait_ge(dma_r14c[i], 16)
                gg = c0
                while gg < c0 + w:
                    gw = GROUP15[gi]
                    vector.bn_stats(
                        out=stats[:, 8, gi, :], in_=r14_buf[:, gg : gg + gw]
                    ).then_inc(dve_stats, 1)
                    ndve += 1
                    gg += gw
                    gi += 1
                c0 += w
            assert gi == len(GROUP15) and ndve == D_R14
            vector.wait_ge(dve_stats, ndve)
            vector.bn_aggr(
                out=mv[:, 8, :], in_=stats[:, 8, 0 : len(GROUP15), :]
            ).then_inc(mv_ready, 1)
            nmv += 1
            assert nmv == MV[14]

            # row 15: chunk-paced
            gi = 0
            c0 = 0
            for i, w in enumerate(R15_CHUNKS):
                vector.wait_ge(dma_rc[i], 16)
                gg = c0
                while gg < c0 + w:
                    gw = GROUP15[gi]
                    vector.bn_stats(
                        out=stats[:, 9, gi, :], in_=r15_buf[:, gg : gg + gw]
                    ).then_inc(dve_stats, 1)
                    ndve += 1
                    gg += gw
                    gi += 1
                c0 += w
            assert gi == len(GROUP15) and ndve == DVE_TOTAL
            vector.wait_ge(dve_stats, ndve)
            vector.bn_aggr(
                out=mv[:, 9, :], in_=stats[:, 9, 0 : len(GROUP15), :]
            ).then_inc(mv_ready, 1)
            nmv += 1
            assert nmv == MV[15]

        @block.scalar
        def _(scalar):
            A = 1.0 / np.sqrt(float(N) * (N - 1))
            cact = 0
            nacc = 0

            scalar.wait_ge(warm_done, 1)
            scalar.activation(
                out=warm[:, 0:1],
                in_=warm[:, 1:2],
                func=mybir.ActivationFunctionType.Copy,
            )

            def acc_pass(a):
                nonlocal nacc
                scalar.wait_ge(dma_a[a], 16)
                for r in range(2):
                    row = xtA[:, a, r * N : (r + 1) * N]
                    scalar.activation(
                        out=row,
                        in_=row,
                        func=mybir.ActivationFunctionType.Copy,
                        accum_out=acc[:, a, r, 0:1],
                    ).then_inc(act_stats, 1)
                    nacc += 1
                    scalar.wait_ge(act_stats, nacc)
                    scalar.activation(
                        out=row,
                        in_=row,
                        func=mybir.ActivationFunctionType.Square,
                        accum_out=acc[:, a, r, 1:2],
                    ).then_inc(act_stats, 1)
                    nacc += 1
                assert nacc == ACTS[a]

            def mdst(u):
                return MS[:, 0, u : u + 1]

            def sdst(u):
                return MS[:, 1, u : u + 1]

            def epi_acc(a):
                nonlocal cact
                scalar.wait_ge(act_stats, ACTS[a])
                for r in range(2):
                    u = 2 * a + r
                    scalar.activation(
                        out=mdst(u),
                        in_=acc[:, a, r, 0:1],
                        func=mybir.ActivationFunctionType.Copy,
                        scale=1.0 / N,
                    ).then_inc(act_done, 1)
                    scalar.activation(
                        out=acc[:, a, r, 2:3],
                        in_=acc[:, a, r, 0:1],
                        func=mybir.ActivationFunctionType.Square,
                        scale=A,
                    ).then_inc(act_done, 1)
                    cact += 2
                    scalar.wait_ge(act_done, cact)
                    scalar.activation(
                        out=acc[:, a, r, 2:3],
                        in_=acc[:, a, r, 2:3],
                        func=mybir.ActivationFunctionType.Copy,
                        scale=-1.0,
                    ).then_inc(act_done, 1)
                    cact += 1
                    scalar.wait_ge(act_done, cact)
                    scalar.activation(
                        out=sdst(u),
                        in_=acc[:, a, r, 1:2],
                        func=mybir.ActivationFunctionType.Sqrt,
                        scale=1.0 / (N - 1),
                        bias=acc[:, a, r, 2:3],
                    ).then_inc(act_done, 1)
                    cact += 1
                assert cact == actd_acc[a]

            def epi_mv(u):
                nonlocal cact
                scalar.wait_ge(mv_ready, MV[u])
                si = u - 6
                scalar.copy(out=mdst(u), in_=mv[:, si, 0:1]).then_inc(act_done, 1)
                scalar.activation(
                    out=sdst(u),
                    in_=mv[:, si, 1:2],
                    func=mybir.ActivationFunctionType.Sqrt,
                    scale=float(N) / (N - 1),
                ).then_inc(act_done, 1)
                cact += 2
                assert cact == actd_mv[u]

            def acc_pass_r13():
                # consume row 13 (second half of the d3 pair buffer)
                nonlocal nacc
                scalar.wait_ge(dma_d[3], 16)
                row = dbuf(3)[:, N : 2 * N]
                scalar.activation(
                    out=row,
                    in_=row,
                    func=mybir.ActivationFunctionType.Copy,
                    accum_out=acc[:, 3, 0, 0:1],
                ).then_inc(act_stats, 1)
                nacc += 1
                scalar.wait_ge(act_stats, nacc)
                scalar.activation(
                    out=row,
                    in_=row,
                    func=mybir.ActivationFunctionType.Square,
                    accum_out=acc[:, 3, 0, 1:2],
                ).then_inc(act_stats, 1)
                nacc += 1
                assert nacc == ACTS_R13

            def epi_acc_r13():
                nonlocal cact
                scalar.wait_ge(act_stats, ACTS_R13)
                scalar.activation(
                    out=mdst(13),
                    in_=acc[:, 3, 0, 0:1],
                    func=mybir.ActivationFunctionType.Copy,
                    scale=1.0 / N,
                ).then_inc(act_done, 1)
                scalar.activation(
                    out=acc[:, 3, 0, 2:3],
                    in_=acc[:, 3, 0, 0:1],
                    func=mybir.ActivationFunctionType.Square,
                    scale=A,
                ).then_inc(act_done, 1)
                cact += 2
                scalar.wait_ge(act_done, cact)
                scalar.activation(
                    out=acc[:, 3, 0, 2:3],
                    in_=acc[:, 3, 0, 2:3],
                    func=mybir.ActivationFunctionType.Copy,
                    scale=-1.0,
                ).then_inc(act_done, 1)
                cact += 1
                scalar.wait_ge(act_done, cact)
                scalar.activation(
                    out=sdst(13),
                    in_=acc[:, 3, 0, 1:2],
                    func=mybir.ActivationFunctionType.Sqrt,
                    scale=1.0 / (N - 1),
                    bias=acc[:, 3, 0, 2:3],
                ).then_inc(act_done, 1)
                cact += 1
                assert cact == ACTD_R13

            acc_pass(0)
            acc_pass(1)
            epi_acc(0)
            acc_pass(2)
            epi_acc(1)
            epi_acc(2)
            acc_pass_r13()
            epi_acc_r13()
            for u in range(6, 13):
                epi_mv(u)
            epi_mv(14)
            # early out: mean+std for slots 0..14 (hidden mid-stream)
            scalar.wait_ge(act_done, actd_mv[14])
            out_dma(scalar, out_sem, 0, 0, RPL - 1)
            out_dma(scalar, out_sem, 1, 0, RPL - 1)
            epi_mv(15)
            scalar.wait_ge(act_done, ACT_TOTAL)
            out_dma(scalar, out_sem, 0, RPL - 1, 1)

    return nc


def kernel(f_vol: np.ndarray) -> np.ndarray:
    from concourse.bass_utils import run_bass_kernel_spmd

    if "nc" not in _CACHE:
        _CACHE["nc"] = _build()
    nc = _CACHE["nc"]

    f_vol = np.ascontiguousarray(f_vol, dtype=np.float32)
    in_maps = [
        {"f_vol": f_vol[i * B_LOCAL : (i + 1) * B_LOCAL]} for i in range(N_CORES)
    ]
    res = run_bass_kernel_spmd(nc, in_maps, core_ids=list(range(N_CORES)))
    return np.concatenate([res.results[i]["out"] for i in range(N_CORES)], axis=0)


# revision 36
# speedup vs baseline: 1.1669x; 1.1669x over previous
"""AdaIN statistics kernel for TRN2, SPMD across 8 NeuronCores. v8.

Input : f_vol [32, 512, 64, 64] f32
Output: [32, 1024] f32 = concat([mean over (h,w), unbiased std over (h,w)], axis=-1)

Sharding: data-parallel over batch - each of the 8 cores handles 4 batches
([4, 512, 64, 64] shard, 32 MiB). No collectives; the host concatenates the
8 per-core [4, 1024] outputs.

DMA facts established by v2-v7 traces on this part:
  - Per-engine stream rate depends on descriptor size: ~26.5 B/ns at
    32 KiB/partition, ~20.6 at 8 KiB, ~14 with sub-128-partition DMAs.
    So the bulk must ride 32 KiB-descriptor full-128-lane slabs.
  - SDMA engine column 15 runs ~21 B/ns whenever all 16 are loaded (both
    HWDGE queues), so its 1/16 byte share sets the stream end (~106 us).
    No descriptor layout shifts bytes off it without the small-DMA
    penalty (v5), so the kernel accepts its pace.
  - Whole-slab completion semaphores fire only when engine 15 drains the
    slab, so slab-gated compute lags the stream.  Pair-slabs complete at
    ~11.9 us spacing vs ~10.9 us of DVE work per pair - DVE keeps pace
    at slab granularity; only the last two rows need finer chunking so
    the exposed tail is one 128-col bn_stats + bn_aggr + 2 ACT ops + 2
    tiny out-DMAs (mean from scalar, std from sync, in parallel).

Lane-major deal: lane p owns rows 16p..16p+15 (row = b*512+c of the
[2048, 4096] row-major view); slot u = output column 16*(p%32)+u of
batch p//32.  ACT consumes slots 0-5 as three whole pair-slabs
(Copy/Square+accumulate); DVE consumes slots 6-13 as four pair-slabs,
slot 14 as one single-row DMA (16 KiB descriptors, arrives ~8 us before
stream end, stats hidden), slot 15 chunked 2048/1024/512/384/128.

Outputs accumulate in MS[128, 2, 16]; the whole output is 4 DMAs: early
mean/std for slots 0..14 (hidden mid-stream), final mean/std for slot 15.

Semaphore discipline (v2 lesson): same-engine RAW through SBUF is NOT
covered by program order; every producer->consumer edge carries an
explicit semaphore observation.  Semaphores and stats buffers are never
reused.  SBUF reuse: DVE pair d3 lands in ACT slab 0's buffer and rows
14/15 in ACT slab 1's, gated on act_stats (ACT's reads retired).
"""

from contextlib import ExitStack

import numpy as np

B, C, H, W = 32, 512, 64, 64
N_CORES = 8
B_LOCAL = B // N_CORES  # 4
N = H * W  # 4096
P = 128
ROWS = B_LOCAL * C  # 2048
RPL = ROWS // P  # 16 rows (slots) per lane

NACT = 3  # ACT pair-slabs: slots (0,1), (2,3), (4,5)
NDVE = 4  # DVE pair-slabs: slots (6,7), (8,9), (10,11), (12,13)
R15_CHUNKS = [2048, 1024, 512, 384, 128]
GROUP15 = [512] * 7 + [384, 128]
# row 14 only needs coarse chunks (it is not the final row): 8 KiB
# descriptors keep line rate and DVE does 8 stats instead of 9
R14_CHUNKS = [2048, 2048]
assert sum(R15_CHUNKS) == N and sum(GROUP15) == N and sum(R14_CHUNKS) == N

_CACHE = {}


def _build():
    import concourse.bass as bass
    from concourse import mybir

    nc = bass.Bass()
    x_ext = nc.declare_dram_parameter(
        "f_vol", [B_LOCAL, C, H, W], mybir.dt.float32, isOutput=False
    )
    out_ext = nc.declare_dram_parameter(
        "out", [B_LOCAL, 2 * C], mybir.dt.float32, isOutput=True
    )

    xl = (
        x_ext.ap()
        .rearrange("b c h w -> (b c) (h w)")
        .rearrange("(p u) f -> p (u f)", u=RPL)
    )

    # DVE cumulative bn_stats after each unit: pairs d0-d2 16 each; d3 is
    # split (DVE row 12 only, 8 stats - ACT consumes row 13 so DVE's
    # terminal backlog shrinks by ~5 us); r14 8; r15 9.
    D_PAIR = {0: 16, 1: 32, 2: 48, 3: 56}
    D_R14 = 64  # row 14 chunked coarse: 8 groups
    DVE_TOTAL = 73
    # mv_ready: rows 6..12 -> 1..7 (row 13 is acc-form), r14 -> 8, r15 -> 9
    MV = {u: u - 5 for u in range(6, 13)}
    MV[14] = 8
    MV[15] = 9
    # act_stats: acc pairs 0-2 -> 4, 8, 12; row-13 pass -> 14
    ACTS = {a: 4 * (a + 1) for a in range(NACT)}
    ACTS_R13 = 14

    with ExitStack() as ctx:
        block = ctx.enter_context(nc.Block(no_gpsimd_drain=True))
        dma_a = [ctx.enter_context(nc.semaphore(f"dma_a{a}")) for a in range(NACT)]
        dma_d = [ctx.enter_context(nc.semaphore(f"dma_d{k}")) for k in range(NDVE)]
        dma_r14c = [
            ctx.enter_context(nc.semaphore(f"dma_r14c{i}"))
            for i in range(len(R14_CHUNKS))
        ]
        dma_rc = [
            ctx.enter_context(nc.semaphore(f"dma_r15c{i}"))
            for i in range(len(R15_CHUNKS))
        ]
        out_sem = ctx.enter_context(nc.semaphore("out_sem"))
        fin_sem = ctx.enter_context(nc.semaphore("fin_sem"))
        dve_stats = ctx.enter_context(nc.semaphore("dve_stats"))
        mv_ready = ctx.enter_context(nc.semaphore("mv_ready"))
        act_stats = ctx.enter_context(nc.semaphore("act_stats"))
        act_done = ctx.enter_context(nc.semaphore("act_done"))
        warm_done = ctx.enter_context(nc.semaphore("warm_done"))

        xtA = ctx.enter_context(
            nc.sbuf_tensor("xtA", [P, NACT, 2 * N], mybir.dt.float32)
        )
        xtD = ctx.enter_context(nc.sbuf_tensor("xtD", [P, 3, 2 * N], mybir.dt.float32))
        stats = ctx.enter_context(
            nc.sbuf_tensor("stats", [P, 10, 9, 6], mybir.dt.float32)
        )
        mv = ctx.enter_context(nc.sbuf_tensor("mv", [P, 10, 2], mybir.dt.float32))
        MS = ctx.enter_context(nc.sbuf_tensor("MS", [P, 2, RPL], mybir.dt.float32))
        acc = ctx.enter_context(
            nc.sbuf_tensor("acc", [P, NACT + 1, 2, 3], mybir.dt.float32)
        )
        warm = ctx.enter_context(nc.sbuf_tensor("warm", [P, 2], mybir.dt.float32))

        # DVE pair k buffer: k<3 -> xtD slot k; k=3 -> xtA slot 0 (reused).
        # Row 14 -> xtA slot 1 low half; row 15 -> xtA slot 1 high half.
        def dbuf(k):
            return xtD[:, k, :] if k < 3 else xtA[:, 0, :]

        r14_buf = xtA[:, 1, 0:N]
        r15_buf = xtA[:, 1, N : 2 * N]

        # act_done gates, ACT emission order: acc epis 0..2 (8 each), row-13
        # epi (4), mv epis 6..12 (2 each), mv epi 14, early out, mv epi 15
        actd_acc = {a: 8 * (a + 1) for a in range(NACT)}
        ACTD_R13 = 28
        actd_mv = {u: 28 + 2 * (u - 5) for u in range(6, 13)}  # 30..42
        actd_mv[14] = 44
        actd_mv[15] = 46
        ACT_TOTAL = actd_mv[15]

        def out_dma(eng, sem, is_std, u0, w):
            dst = bass.AP(
                tensor=out_ext,
                offset=is_std * C + u0,
                ap=[[2 * C, B_LOCAL], [RPL, P // B_LOCAL], [1, w]],
            )
            with nc.allow_non_contiguous_dma(reason="4B-per-lane stat column"):
                eng.dma_start(out=dst, in_=MS[:, is_std, u0 : u0 + w]).then_inc(
                    sem, 16
                )

        @block.sync
        def _(sync):
            def pair_src(s0):  # slots (s0, s0+1)
                return xl[:, s0 * N : (s0 + 2) * N]

            # interleave ACT and DVE pair-slabs
            sync.dma_start(out=xtA[:, 0, :], in_=pair_src(0)).then_inc(dma_a[0], 16)
            sync.dma_start(out=dbuf(0), in_=pair_src(6)).then_inc(dma_d[0], 16)
            sync.dma_start(out=xtA[:, 1, :], in_=pair_src(2)).then_inc(dma_a[1], 16)
            sync.dma_start(out=dbuf(1), in_=pair_src(8)).then_inc(dma_d[1], 16)
            sync.dma_start(out=xtA[:, 2, :], in_=pair_src(4)).then_inc(dma_a[2], 16)
            sync.dma_start(out=dbuf(2), in_=pair_src(10)).then_inc(dma_d[2], 16)
            # d3 reuses ACT slab 0's buffer; ACT finished reading it at
            # act_stats >= 4
            sync.wait_ge(act_stats, ACTS[0])
            sync.dma_start(out=dbuf(3), in_=pair_src(12)).then_inc(dma_d[3], 16)
            # rows 14/15 reuse ACT slab 1's buffer
            sync.wait_ge(act_stats, ACTS[1])
            c0 = 0
            for i, w in enumerate(R14_CHUNKS):
                sync.dma_start(
                    out=r14_buf[:, c0 : c0 + w],
                    in_=xl[:, 14 * N + c0 : 14 * N + c0 + w],
                ).then_inc(dma_r14c[i], 16)
                c0 += w
            c0 = 0
            for i, w in enumerate(R15_CHUNKS):
                sync.dma_start(
                    out=r15_buf[:, c0 : c0 + w],
                    in_=xl[:, 15 * N + c0 : 15 * N + c0 + w],
                ).then_inc(dma_rc[i], 16)
                c0 += w
            # std outs on the sync queue (mean outs ride the scalar queue)
            # so neither final queues behind both early-out descriptor sets
            sync.wait_ge(act_done, actd_mv[14])
            out_dma(sync, fin_sem, 1, 0, RPL - 1)
            sync.wait_ge(act_done, ACT_TOTAL)
            out_dma(sync, fin_sem, 1, RPL - 1, 1)
            sync.wait_ge(out_sem, 16 * 2)
            sync.wait_ge(fin_sem, 16 * 2)

        @block.vector
        def _(vector):
            ndve = 0
            nmv = 0

            vector.memset(warm[:, :], 0.0).then_inc(warm_done, 1)

            for k in range(NDVE):
                rows = (0, 1) if k < 3 else (0,)  # d3 row 1 goes to ACT
                vector.wait_ge(dma_d[k], 16)
                buf = dbuf(k)
                for r in rows:
                    si = 2 * k + r  # stats index for slot u = 6+2k+r
                    for g in range(8):
                        vector.bn_stats(
                            out=stats[:, si, g, :],
                            in_=buf[:, (r * 8 + g) * 512 : (r * 8 + g + 1) * 512],
                        ).then_inc(dve_stats, 1)
                        ndve += 1
                assert ndve == D_PAIR[k]
                vector.wait_ge(dve_stats, ndve)
                for r in rows:
                    si = 2 * k + r
                    vector.bn_aggr(
                        out=mv[:, si, :], in_=stats[:, si, 0:8, :]
                    ).then_inc(mv_ready, 1)
                    nmv += 1
                assert nmv == MV[6 + 2 * k + rows[-1]]

            # row 14: chunk-paced, uniform 512-col groups (4 per chunk)
            gi = 0
            c0 = 0
            for i, w in enumerate(R14_CHUNKS):
                vector.wait_ge(dma_r14c[i], 16)
                for _ in range(w // 512):
                    vector.bn_stats(
                        out=stats[:, 8, gi, :],
                        in_=r14_buf[:, gi * 512 : (gi + 1) * 512],
                    ).then_inc(dve_stats, 1)
                    ndve += 1
                    gi += 1
                c0 += w
            assert gi == 8 and ndve == D_R14
            vector.wait_ge(dve_stats, ndve)
            vector.bn_aggr(out=mv[:, 8, :], in_=stats[:, 8, 0:8, :]).then_inc(
                mv_ready, 1
            )
            nmv += 1
            assert nmv == MV[14]

            # row 15: chunk-paced
            gi = 0
            c0 = 0
            for i, w in enumerate(R15_CHUNKS):
                vector.wait_ge(dma_rc[i], 16)
                gg = c0
                while gg < c0 + w:
                    gw = GROUP15[gi]
                    vector.bn_stats(
                        out=stats[:, 9, gi, :], in_=r15_buf[:, gg : gg + gw]
                    ).then_inc(dve_stats, 1)
                    ndve += 1
                    gg += gw
                    gi += 1
                c0 += w
            assert gi == len(GROUP15) and ndve == DVE_TOTAL
            vector.wait_ge(dve_stats, ndve)
            vector.bn_aggr(
                out=mv[:, 9, :], in_=stats[:, 9, 0 : len(GROUP15), :]
            ).then_inc(mv_ready, 1)
            nmv += 1
            assert nmv == MV[15]

        @block.scalar
        def _(scalar):
            A = 1.0 / np.sqrt(float(N) * (N - 1))
            cact = 0
            nacc = 0

            scalar.wait_ge(warm_done, 1)
            scalar.activation(
                out=warm[:, 0:1],
                in_=warm[:, 1:2],
                func=mybir.ActivationFunctionType.Copy,
            )

            def acc_pass(a):
                nonlocal nacc
                scalar.wait_ge(dma_a[a], 16)
                for r in range(2):
                    row = xtA[:, a, r * N : (r + 1) * N]
                    scalar.activation(
                        out=row,
                        in_=row,
                        func=mybir.ActivationFunctionType.Copy,
                        accum_out=acc[:, a, r, 0:1],
                    ).then_inc(act_stats, 1)
                    nacc += 1
                    scalar.wait_ge(act_stats, nacc)
                    scalar.activation(
                        out=row,
                        in_=row,
                        func=mybir.ActivationFunctionType.Square,
                        accum_out=acc[:, a, r, 1:2],
                    ).then_inc(act_stats, 1)
                    nacc += 1
                assert nacc == ACTS[a]

            def mdst(u):
                return MS[:, 0, u : u + 1]

            def sdst(u):
                return MS[:, 1, u : u + 1]

            def epi_acc(a):
                nonlocal cact
                scalar.wait_ge(act_stats, ACTS[a])
                for r in range(2):
                    u = 2 * a + r
                    scalar.activation(
                        out=mdst(u),
                        in_=acc[:, a, r, 0:1],
                        func=mybir.ActivationFunctionType.Copy,
                        scale=1.0 / N,
                    ).then_inc(act_done, 1)
                    scalar.activation(
                        out=acc[:, a, r, 2:3],
                        in_=acc[:, a, r, 0:1],
                        func=mybir.ActivationFunctionType.Square,
                        scale=A,
                    ).then_inc(act_done, 1)
                    cact += 2
                    scalar.wait_ge(act_done, cact)
                    scalar.activation(
                        out=acc[:, a, r, 2:3],
                        in_=acc[:, a, r, 2:3],
                        func=mybir.ActivationFunctionType.Copy,
                        scale=-1.0,
                    ).then_inc(act_done, 1)
                    cact += 1
                    scalar.wait_ge(act_done, cact)
                    scalar.activation(
                        out=sdst(u),
                        in_=acc[:, a, r, 1:2],
                        func=mybir.ActivationFunctionType.Sqrt,
                        scale=1.0 / (N - 1),
                        bias=acc[:, a, r, 2:3],
                    ).then_inc(act_done, 1)
                    cact += 1
                assert cact == actd_acc[a]

            def epi_mv(u):
                nonlocal cact
                scalar.wait_ge(mv_ready, MV[u])
                si = u - 6
                scalar.copy(out=mdst(u), in_=mv[:, si, 0:1]).then_inc(act_done, 1)
                scalar.activation(
                    out=sdst(u),
                    in_=mv[:, si, 1:2],
                    func=mybir.ActivationFunctionType.Sqrt,
                    scale=float(N) / (N - 1),
                ).then_inc(act_done, 1)
                cact += 2
                assert cact == actd_mv[u]

            def acc_pass_r13():
                # consume row 13 (second half of the d3 pair buffer)
                nonlocal nacc
                scalar.wait_ge(dma_d[3], 16)
                row = dbuf(3)[:, N : 2 * N]
                scalar.activation(
                    out=row,
                    in_=row,
                    func=mybir.ActivationFunctionType.Copy,
                    accum_out=acc[:, 3, 0, 0:1],
                ).then_inc(act_stats, 1)
                nacc += 1
                scalar.wait_ge(act_stats, nacc)
                scalar.activation(
                    out=row,
                    in_=row,
                    func=mybir.ActivationFunctionType.Square,
                    accum_out=acc[:, 3, 0, 1:2],
                ).then_inc(act_stats, 1)
                nacc += 1
                assert nacc == ACTS_R13

            def epi_acc_r13():
                nonlocal cact
                scalar.wait_ge(act_stats, ACTS_R13)
                scalar.activation(
                    out=mdst(13),
                    in_=acc[:, 3, 0, 0:1],
                    func=mybir.ActivationFunctionType.Copy,
                    scale=1.0 / N,
                ).then_inc(act_done, 1)
                scalar.activation(
                    out=acc[:, 3, 0, 2:3],
                    in_=acc[:, 3, 0, 0:1],
                    func=mybir.ActivationFunctionType.Square,
                    scale=A,
                ).then_inc(act_done, 1)
                cact += 2
                scalar.wait_ge(act_done, cact)
                scalar.activation(
                    out=acc[:, 3, 0, 2:3],
                    in_=acc[:, 3, 0, 2:3],
                    func=mybir.ActivationFunctionType.Copy,
                    scale=-1.0,
                ).then_inc(act_done, 1)
                cact += 1
                scalar.wait_ge(act_done, cact)
                scalar.activation(
                    out=sdst(13),
                    in_=acc[:, 3, 0, 1:2],
                    func=mybir.ActivationFunctionType.Sqrt,
                    scale=1.0 / (N - 1),
                    bias=acc[:, 3, 0, 2:3],
                ).then_inc(act_done, 1)
                cact += 1
                assert cact == ACTD_R13

            acc_pass(0)
            acc_pass(1)
            epi_acc(0)
            acc_pass(2)
            epi_acc(1)
            epi_acc(2)
            acc_pass_r13()
            epi_acc_r13()
            for u in range(6, 13):
                epi_mv(u)
            epi_mv(14)
            # early mean out for slots 0..14 (early std goes via sync's queue)
            scalar.wait_ge(act_done, actd_mv[14])
            out_dma(scalar, out_sem, 0, 0, RPL - 1)
            epi_mv(15)
            scalar.wait_ge(act_done, ACT_TOTAL)
            out_dma(scalar, out_sem, 0, RPL - 1, 1)

    return nc


def kernel(f_vol: np.ndarray) -> np.ndarray:
    from concourse.bass_utils import run_bass_kernel_spmd

    if "nc" not in _CACHE:
        _CACHE["nc"] = _build()
    nc = _CACHE["nc"]

    f_vol = np.ascontiguousarray(f_vol, dtype=np.float32)
    in_maps = [
        {"f_vol": f_vol[i * B_LOCAL : (i + 1) * B_LOCAL]} for i in range(N_CORES)
    ]
    res = run_bass_kernel_spmd(nc, in_maps, core_ids=list(range(N_CORES)))
    return np.concatenate([res.results[i]["out"] for i in range(N_CORES)], axis=0)
